# revision 31
# baseline (speedup 1.0000x reference)
"""Trainium2 Bass kernel: causal MHA block (B=2, S=2048, D=4096, 32 heads x 128,
fp32 in/out, interleaved RoPE), tensor-parallel over heads on 8 NeuronCores with
a per-batch AllToAll into a sequence-parallel output projection.

v3: bf16 streams, merged single-pass QKV (m-major wq/wk so the first chain only
needs one DMA chunk), V resident in SBUF, per-batch q/k DRAM round-trip,
paired-bank score tiles (one exp per 1024 cols), score/PV weave with the
denominator-reciprocal (bf16) covered by the PV chain, head-ahead q/k prefetch
on the scalar HWDGE queue, at_sb prefetch on gpsimd right after each AllToAll,
early wo loads for the first WO block.
"""

import sys

if "/opt/trn_rl_repo" not in sys.path:
    sys.path.insert(0, "/opt/trn_rl_repo")

import numpy as np
import ml_dtypes

import concourse.bass as bass
import concourse.tile as tile
from concourse import bacc, mybir
from concourse.bass_utils import run_bass_kernel_spmd

F32 = mybir.dt.float32
F32R = mybir.dt.float32r
BF16 = mybir.dt.bfloat16

B, S, D = 2, 2048, 4096
H, HD = 32, 128
NCORES = 8
HPC = H // NCORES        # 4 heads per core
F = HPC * HD             # 512 features per core
TOK = B * S              # 4096 tokens
KT = D // 128            # 32 contraction tiles
NB = TOK // 256          # 16 token blocks of 256
SBLK = S // 128          # 16 k-blocks of 128 per sequence
SCALE = 1.0 / float(np.sqrt(HD))
NEG = -1e30

_CACHE = {}


def _build():
    nc = bacc.Bacc("TRN2", target_bir_lowering=False, debug=False,
                   num_devices=NCORES)

    x_d = nc.dram_tensor("xt", [NB, 128, KT * 256], BF16, kind="ExternalInput")
    # wq/wk m-major: [128, HPC, KT*128]; wv kt-major: [128, KT*F]
    wq_d = nc.dram_tensor("wqT", [128, HPC * KT * 128], BF16,
                          kind="ExternalInput")
    wk_d = nc.dram_tensor("wkT", [128, HPC * KT * 128], BF16,
                          kind="ExternalInput")
    wv_d = nc.dram_tensor("wvT", [128, KT * F], BF16, kind="ExternalInput")
    wo_d = nc.dram_tensor("woT", [D // 512, 4, 128, 8 * 512], BF16,
                          kind="ExternalInput")
    cos_d = nc.dram_tensor("cosE", [128, S], BF16, kind="ExternalInput")
    sin_d = nc.dram_tensor("sinE", [128, S], BF16, kind="ExternalInput")
    mask_d = nc.dram_tensor("masks", [128, 4 * 512], BF16, kind="ExternalInput")
    perm_d = nc.dram_tensor("permT", [128, 128], BF16, kind="ExternalInput")
    onesb_d = nc.dram_tensor("onesb", [128, 128], BF16, kind="ExternalInput")
    out_d = nc.dram_tensor("out", [TOK // NCORES, D], F32, kind="ExternalOutput")

    with tile.TileContext(nc) as tc:
        dram = tc.alloc_tile_pool(name="dram", bufs=1, space="DRAM")
        q_sp = [dram.tile([HPC, 128, S], BF16, name=f"q_sp{b}")
                for b in range(B)]
        k_sp = [dram.tile([HPC, 128, S], BF16, name=f"k_sp{b}")
                for b in range(B)]
        a2a_in = [dram.tile([NCORES, F, 256], BF16, name=f"a2a_in{b}")
                  for b in range(B)]
        a2a_out = [dram.tile([NCORES, F, 256], BF16, name=f"a2a_out{b}")
                   for b in range(B)]

        with tc.tile_pool(name="consts", bufs=1) as cpool:
            perm_sb = cpool.tile([128, 128], BF16)
            nc.gpsimd.dma_start(out=perm_sb[:], in_=perm_d[:, :])
            onesb_sb = cpool.tile([128, 128], BF16)
            nc.gpsimd.dma_start(out=onesb_sb[:], in_=onesb_d[:, :])
            mask_sb = cpool.tile([128, 4 * 512], BF16)
            nc.gpsimd.dma_start(out=mask_sb[:], in_=mask_d[:, :])
            cos_sb = cpool.tile([128, S], BF16)
            nc.gpsimd.dma_start(out=cos_sb[:], in_=cos_d[:, :])
            sin_sb = cpool.tile([128, S], BF16)
            nc.gpsimd.dma_start(out=sin_sb[:], in_=sin_d[:, :])
            # v for both batches, all 4 heads: [128 tok-part, 32 blk * 512]
            v_all = cpool.tile([128, (TOK // 128) * F], BF16, name="v_all")

            # q/k tiles for the very first attention head, loaded mid-QKV
            q0_sb = cpool.tile([128, S], BF16, tag="q0", name="q0_sb")
            k0_sb = cpool.tile([128, S], BF16, tag="k0", name="k0_sb")

            # ======== merged QKV phase: one pass over x, weights resident
            with tc.tile_pool(name="wpool", bufs=1) as wpool, \
                 tc.tile_pool(name="xpool", bufs=2) as xpool, \
                 tc.tile_pool(name="qkps", bufs=4, space="PSUM") as qkps, \
                 tc.tile_pool(name="vps", bufs=2, space="PSUM") as vps, \
                 tc.tile_pool(name="rotps", bufs=2, space="PSUM") as rotps, \
                 tc.tile_pool(name="rwork", bufs=2) as rwork:

                w_sb = {}
                for nm, wd in (("q", wq_d), ("k", wk_d)):
                    t = wpool.tile([128, HPC * KT * 128], BF16, tag=f"w{nm}",
                                   name=f"w_{nm}")
                    ml = KT * 128
                    for m in range(HPC):
                        nc.scalar.dma_start(
                            out=t[:, m * ml:(m + 1) * ml],
                            in_=wd[:, m * ml:(m + 1) * ml])
                    w_sb[nm] = t
                wv_sb = wpool.tile([128, KT * F], BF16, tag="wv", name="w_v")
                chunk = KT * F // 8
                for c in range(8):
                    nc.scalar.dma_start(
                        out=wv_sb[:, c * chunk:(c + 1) * chunk],
                        in_=wv_d[:, c * chunk:(c + 1) * chunk])

                pending = []   # delayed rope: (raw, o_sp, m, nb)

                def emit_rope(item):
                    raw, o_sp, m, nb = item
                    rot = rotps.tile([128, 256], F32, tag="rot", name="rot")
                    nc.tensor.matmul(rot[:], perm_sb[:], raw[:],
                                     start=True, stop=True)
                    c0 = (nb % (S // 256)) * 256
                    t1 = rwork.tile([128, 256], BF16, tag="t1", name="t1")
                    nc.vector.tensor_mul(t1[:], raw[:], cos_sb[:, c0:c0 + 256])
                    t2 = rwork.tile([128, 256], BF16, tag="t2", name="t2")
                    nc.vector.tensor_mul(t2[:], rot[:], sin_sb[:, c0:c0 + 256])
                    qf = rwork.tile([128, 256], BF16, tag="qf", name="qf")
                    nc.vector.tensor_add(qf[:], t1[:], t2[:])
                    b, pos = (nb * 256) // S, (nb * 256) % S
                    nc.sync.dma_start(
                        out=o_sp[b][m, :, pos:pos + 256], in_=qf[:])

                for nb in range(NB):
                    xh = xpool.tile([128, KT * 256], BF16, tag="xh", name="xh")
                    xc = KT * 256 // 4
                    for c in range(4):
                        nc.sync.dma_start(
                            out=xh[:, c * xc:(c + 1) * xc],
                            in_=x_d[nb, :, c * xc:(c + 1) * xc])
                    for qk in range(2):
                        w_t = w_sb["q"] if qk == 0 else w_sb["k"]
                        o_sp = q_sp if qk == 0 else k_sp
                        for m in range(HPC):
                            ps = qkps.tile([128, 256], F32, tag="ps", name="ps")
                            for kt in range(KT):
                                nc.tensor.matmul(
                                    ps[:],
                                    w_t[:, (m * KT + kt) * 128:
                                        (m * KT + kt + 1) * 128],
                                    xh[:, kt * 256:(kt + 1) * 256],
                                    start=(kt == 0), stop=(kt == KT - 1))
                            raw = rwork.tile([128, 256], BF16, tag="raw",
                                             name="raw")
                            nc.scalar.copy(raw[:], ps[:])
                            pending.append((raw, o_sp, m, nb))
                            if len(pending) > 1:
                                emit_rope(pending.pop(0))
                    for mt in range(2):
                        vp = vps.tile([128, F], F32, tag="vp", name="vp")
                        for kt in range(KT):
                            nc.tensor.matmul(
                                vp[:],
                                xh[:, kt * 256 + mt * 128:
                                   kt * 256 + (mt + 1) * 128],
                                wv_sb[:, kt * F:(kt + 1) * F],
                                start=(kt == 0), stop=(kt == KT - 1))
                        st = nb * 2 + mt
                        nc.scalar.copy(v_all[:, st * F:(st + 1) * F], vp[:])
                    if nb == NB // 2 - 1:
                        # batch 0 q/k complete: prefetch head (0,0)
                        nc.scalar.dma_start(out=q0_sb[:],
                                            in_=q_sp[0][0, :, :])
                        nc.scalar.dma_start(out=k0_sb[:],
                                            in_=k_sp[0][0, :, :])
                while pending:
                    emit_rope(pending.pop(0))

            # ======== attention (woven) + per-batch AllToAll
            with tc.tile_pool(name="aqk", bufs=2) as apool, \
                 tc.tile_pool(name="exw", bufs=17) as expool, \
                 tc.tile_pool(name="esw", bufs=17) as espool, \
                 tc.tile_pool(name="amisc", bufs=2) as misc, \
                 tc.tile_pool(name="atprep", bufs=1) as atprep, \
                 tc.tile_pool(name="wo0p", bufs=1) as wo0p:

                at4 = [[atprep.tile([128, 8 * 256], BF16, tag=f"at4_{b}_{j}",
                                    name=f"at4_{b}_{j}")
                        for j in range(4)] for b in range(B)]
                wo0 = [wo0p.tile([128, 8 * 512], BF16, tag=f"wo0_{q}",
                                 name=f"wo0_{q}") for q in range(2)]

                heads = [(b, h) for b in range(B) for h in range(HPC)]
                tiles = {(0, 0): (q0_sb, k0_sb)}

                def emit_qk_load(bh):
                    b, h = bh
                    q_sb = apool.tile([128, S], BF16, tag="q", name="q_sb")
                    nc.scalar.dma_start(out=q_sb[:], in_=q_sp[b][h, :, :])
                    k_sb = apool.tile([128, S], BF16, tag="k", name="k_sb")
                    nc.scalar.dma_start(out=k_sb[:], in_=k_sp[b][h, :, :])
                    tiles[bh] = (q_sb, k_sb)

                with tc.tile_pool(name="scps", bufs=2, space="PSUM") as scps, \
                     tc.tile_pool(name="pvps", bufs=2, space="PSUM") as pvps, \
                     tc.tile_pool(name="dps", bufs=2, space="PSUM") as dps:

                    finq = []

                    def drain(k):
                        for _ in range(min(k, len(finq))):
                            finq.pop(0)()

                    def build_fins(b, h, qt, exs, ess):
                        nkt = len(exs)
                        nds = len(ess)
                        st8 = b * SBLK
                        state = {}

                        def f_ds(i):
                            def go():
                                if i == 0:
                                    state["ds"] = dps.tile(
                                        [1, 512], F32, tag="dsum", name="dsum")
                                nc.tensor.matmul(
                                    state["ds"][:], onesb_sb[:, 0:1], ess[i],
                                    start=(i == 0), stop=(i == nds - 1))
                                if i == nds - 1:
                                    rec = misc.tile([1, 512], BF16, tag="rec",
                                                    name="rec")
                                    with nc.allow_low_precision(
                                            reason="softmax denom bf16"):
                                        nc.vector.reciprocal(
                                            rec[:], state["ds"][:])
                                    state["rec"] = rec
                            return go

                        def f_pv(i):
                            def go():
                                if i == 0:
                                    state["pv"] = pvps.tile(
                                        [128, 512], F32, tag="pv", name="pv")
                                blk = (st8 + i) * F + h * 128
                                nc.tensor.matmul(
                                    state["pv"][:],
                                    v_all[:, blk:blk + 128], exs[i],
                                    start=(i == 0), stop=(i == nkt - 1))
                            return go

                        def f_bc():
                            def go():
                                bc = dps.tile([128, 512], F32, tag="dsum",
                                              name="bc")
                                nc.tensor.matmul(bc[:], onesb_sb[0:1, :],
                                                 state["rec"][:],
                                                 start=True, stop=True)
                                bcs = misc.tile([128, 512], F32, tag="bcs",
                                                name="bcs")
                                nc.scalar.copy(bcs[:], bc[:])
                                state["bcs"] = bcs
                            return go

                        def f_at():
                            def go():
                                at = misc.tile([128, 512], BF16, tag="at",
                                               name="at")
                                nc.vector.tensor_mul(
                                    at[:], state["pv"][:], state["bcs"][:])
                                for u in range(2):
                                    nc.sync.dma_start(
                                        out=a2a_in[b][2 * qt + u,
                                                      h * 128:(h + 1) * 128, :],
                                        in_=at[:, u * 256:(u + 1) * 256])
                            return go

                        fins = [f_ds(i) for i in range(nds)]
                        fins += [f_pv(i) for i in range(nkt)]
                        fins += [f_bc(), f_at()]
                        return fins

                    def emit_at4(b):
                        for j in range(4):
                            for c in range(8):
                                kt = j * 8 + c
                                jj, off = kt // 4, (kt % 4) * 128
                                nc.gpsimd.dma_start(
                                    out=at4[b][j][:, c * 256:(c + 1) * 256],
                                    in_=a2a_out[b][jj, off:off + 128, :])

                    pace = [0]
                    for hi, (b, h) in enumerate(heads):
                        q_sb, k_sb = tiles.pop((b, h))
                        if hi + 1 < len(heads):
                            emit_qk_load(heads[hi + 1])
                        qts = range(4) if hi % 2 == 0 else range(3, -1, -1)
                        for qt in qts:
                            nkt = 4 * qt + 4
                            exs = []
                            ess = []
                            for kp in range(nkt // 2):
                                sc = scps.tile([128, 1024], F32, tag="sc",
                                               name="sc")
                                for hf in range(2):
                                    kt = 2 * kp + hf
                                    nc.tensor.matmul(
                                        sc[:, hf * 512:(hf + 1) * 512],
                                        k_sb[:, kt * 128:(kt + 1) * 128],
                                        q_sb[:, qt * 512:(qt + 1) * 512],
                                        start=True, stop=True)
                                ex = expool.tile([128, 1024], BF16, tag="ex",
                                                 name="ex")
                                nc.scalar.activation(
                                    ex[:], sc[:],
                                    mybir.ActivationFunctionType.Exp,
                                    scale=SCALE)
                                # causal mask applied post-exp (0/1 multiply)
                                # on Pool, off the mm->exp critical path
                                for hf in range(2):
                                    r = 2 * kp + hf - 4 * qt
                                    if r >= 0:
                                        sl = ex[:, hf * 512:(hf + 1) * 512]
                                        nc.vector.tensor_mul(
                                            sl, sl,
                                            mask_sb[:, r * 512:(r + 1) * 512])
                                exs.append(ex[:, 0:512])
                                exs.append(ex[:, 512:1024])
                                es = espool.tile([128, 512], BF16, tag="es",
                                                 name="es")
                                nc.gpsimd.tensor_add(
                                    es[:], ex[:, 0:512], ex[:, 512:1024])
                                ess.append(es)
                                # global pacing: 68 fins / 20 pairs per head
                                pace[0] += 17
                                nd = pace[0] // 5
                                pace[0] -= nd * 5
                                drain(nd)
                            finq.extend(build_fins(b, h, qt, exs, ess))
                        if b == 1 and h == 1:
                            # at4(b0): Pool reaches this mid-b1, a2a0 done,
                            # no engine stall; lands well before WO(b0)
                            emit_at4(0)
                        if h == HPC - 1:
                            drain(len(finq))
                            pace[0] = 0
                            nc.gpsimd.collective_compute(
                                "AllToAll", mybir.AluOpType.bypass,
                                replica_groups=[list(range(NCORES))],
                                ins=[a2a_in[b][:]], outs=[a2a_out[b][:]])
                            if b == 1:
                                emit_at4(1)
                            if b == 0:
                                # first WO block's weights, no deps: load now
                                for q in range(2):
                                    nc.sync.dma_start(
                                        out=wo0[q][:], in_=wo_d[0, q, :, :])

                # ======== WO: out[tok_slice] = attn @ wo^T, batch 0 then 1
                with tc.tile_pool(name="wop", bufs=3) as wopool, \
                     tc.tile_pool(name="pswo", bufs=1, space="PSUM") as wops, \
                     tc.tile_pool(name="wout", bufs=3) as wout:
                    for b in range(B):
                        for n in range(D // 512):
                            pss = [wops.tile([128, 512], F32, tag=f"pw{mt}",
                                             name=f"ps{mt}") for mt in range(2)]
                            for quad in range(4):
                                if b == 0 and n == 0 and quad < 2:
                                    wo_sb = wo0[quad]
                                else:
                                    wo_sb = wopool.tile([128, 8 * 512], BF16,
                                                        tag="wo", name="wo_sb")
                                    nc.scalar.dma_start(out=wo_sb[:],
                                                        in_=wo_d[n, quad, :, :])
                                for mt in range(2):
                                    for kk in range(8):
                                        kt = quad * 8 + kk
                                        nc.tensor.matmul(
                                            pss[mt][:],
                                            at4[b][kt // 8][
                                                :, (kt % 8) * 256 + mt * 128:
                                                (kt % 8) * 256 + (mt + 1) * 128],
                                            wo_sb[:, kk * 512:(kk + 1) * 512],
                                            start=(kt == 0), stop=(kt == KT - 1))
                            for mt in range(2):
                                o_sb = wout.tile([128, 512], F32, name="o_sb")
                                nc.scalar.copy(o_sb[:], pss[mt][:])
                                nc.sync.dma_start(
                                    out=out_d[b * 256 + mt * 128:
                                              b * 256 + (mt + 1) * 128,
                                              n * 512:(n + 1) * 512],
                                    in_=o_sb[:])

    nc.compile()
    return nc


def _host_inputs(x, wq, wk, wv, wo):
    bf = ml_dtypes.bfloat16
    x = np.asarray(x, dtype=np.float32).reshape(TOK, D)
    xT = x.T                                              # [D, TOK]
    xt = np.ascontiguousarray(
        xT.reshape(KT, 128, NB, 256).transpose(2, 1, 0, 3)
        .reshape(NB, 128, KT * 256)).astype(bf)

    woT = np.asarray(wo, dtype=np.float32).T              # [f_in, d_out]
    wot = np.ascontiguousarray(
        woT.reshape(4, 8, 128, 8, 512).transpose(3, 0, 2, 1, 4)
        .reshape(8, 4, 128, 8 * 512)).astype(bf)

    inv = (1.0 / (10000.0 ** (np.arange(0, HD, 2, dtype=np.float64) / HD)))
    fr = np.outer(np.arange(S, dtype=np.float64), inv)    # [S, HD/2]
    cosE = np.ascontiguousarray(
        np.repeat(np.cos(fr).T, 2, axis=0)).astype(bf)   # [128, S]
    sinE = np.ascontiguousarray(
        np.repeat(np.sin(fr).T, 2, axis=0)).astype(bf)

    masks = np.ones([128, 4 * 512], dtype=np.float32)
    qi = np.arange(512)
    pi = np.arange(128)
    for r in range(4):
        masks[:, r * 512:(r + 1) * 512][qi[None, :] < (r * 128 + pi)[:, None]] = 0.0

    permT = np.zeros([128, 128], dtype=np.float32)
    ii = np.arange(0, 128, 2)
    permT[ii + 1, ii] = -1.0
    permT[ii, ii + 1] = 1.0

    ones = np.ones([128, 128], dtype=np.float32)

    def wprep_m(w, sl):      # m-major: [128, HPC, KT, 128]
        wT = np.asarray(w, np.float32)[sl, :].T           # [D, F]
        return np.ascontiguousarray(
            wT.reshape(KT, 128, HPC, 128).transpose(1, 2, 0, 3)
            .reshape(128, HPC * KT * 128)).astype(bf)

    def wprep_kt(w, sl):     # kt-major: [128, KT, F]
        wT = np.asarray(w, np.float32)[sl, :].T
        return np.ascontiguousarray(
            wT.reshape(KT, 128, F).transpose(1, 0, 2)
            .reshape(128, KT * F)).astype(bf)

    maps = []
    for i in range(NCORES):
        sl = slice(i * F, (i + 1) * F)
        maps.append(dict(
            xt=xt,
            wqT=wprep_m(wq, sl),
            wkT=wprep_m(wk, sl),
            wvT=wprep_kt(wv, sl),
            woT=wot,
            cosE=cosE, sinE=sinE, masks=masks.astype(bf),
            permT=permT.astype(bf), onesb=ones.astype(bf),
        ))
    return maps


def kernel(x, start_pos, wq, wk, wv, wo, _trace=False):
    if "nc" not in _CACHE:
        _CACHE["nc"] = _build()
    nc = _CACHE["nc"]
    maps = _host_inputs(x, wq, wk, wv, wo)
    res = run_bass_kernel_spmd(nc, maps, core_ids=list(range(NCORES)),
                               trace=_trace)
    _CACHE["last"] = res
    full = np.empty([TOK, D], dtype=np.float32)
    for j in range(NCORES):
        o = res.results[j]["out"]
        full[j * 256:(j + 1) * 256] = o[:256]
        full[S + j * 256: S + (j + 1) * 256] = o[256:]
    return full.reshape(B, S, D)


# revision 33
# speedup vs baseline: 1.0245x; 1.0245x over previous
"""Trainium2 Bass kernel: causal MHA block (B=2, S=2048, D=4096, 32 heads x 128,
fp32 in/out, interleaved RoPE), tensor-parallel over heads on 8 NeuronCores with
a per-batch AllToAll into a sequence-parallel output projection.

v3: bf16 streams, merged single-pass QKV (m-major wq/wk so the first chain only
needs one DMA chunk), V resident in SBUF, per-batch q/k DRAM round-trip,
paired-bank score tiles (one exp per 1024 cols), score/PV weave with the
denominator-reciprocal (bf16) covered by the PV chain, head-ahead q/k prefetch
on the scalar HWDGE queue, at_sb prefetch on gpsimd right after each AllToAll,
early wo loads for the first WO block.
"""

import sys

if "/opt/trn_rl_repo" not in sys.path:
    sys.path.insert(0, "/opt/trn_rl_repo")

import numpy as np
import ml_dtypes

import concourse.bass as bass
import concourse.tile as tile
from concourse import bacc, mybir
from concourse.bass_utils import run_bass_kernel_spmd

F32 = mybir.dt.float32
F32R = mybir.dt.float32r
BF16 = mybir.dt.bfloat16

B, S, D = 2, 2048, 4096
H, HD = 32, 128
NCORES = 8
HPC = H // NCORES        # 4 heads per core
F = HPC * HD             # 512 features per core
TOK = B * S              # 4096 tokens
KT = D // 128            # 32 contraction tiles
NB = TOK // 256          # 16 token blocks of 256
SBLK = S // 128          # 16 k-blocks of 128 per sequence
SCALE = 1.0 / float(np.sqrt(HD))
NEG = -1e30

_CACHE = {}


def _build():
    nc = bacc.Bacc("TRN2", target_bir_lowering=False, debug=False,
                   num_devices=NCORES)

    x_d = nc.dram_tensor("xt", [NB, 128, KT * 256], BF16, kind="ExternalInput")
    # wq/wk m-major: [128, HPC, KT*128]; wv kt-major: [128, KT*F]
    wq_d = nc.dram_tensor("wqT", [128, HPC * KT * 128], BF16,
                          kind="ExternalInput")
    wk_d = nc.dram_tensor("wkT", [128, HPC * KT * 128], BF16,
                          kind="ExternalInput")
    wv_d = nc.dram_tensor("wvT", [128, KT * F], BF16, kind="ExternalInput")
    wo_d = nc.dram_tensor("woT", [D // 512, 4, 128, 8 * 512], BF16,
                          kind="ExternalInput")
    cos_d = nc.dram_tensor("cosE", [128, S], BF16, kind="ExternalInput")
    sin_d = nc.dram_tensor("sinE", [128, S], BF16, kind="ExternalInput")
    mask_d = nc.dram_tensor("masks", [128, 4 * 512], BF16, kind="ExternalInput")
    perm_d = nc.dram_tensor("permT", [128, 128], BF16, kind="ExternalInput")
    onesb_d = nc.dram_tensor("onesb", [128, 128], BF16, kind="ExternalInput")
    out_d = nc.dram_tensor("out", [TOK // NCORES, D], F32, kind="ExternalOutput")

    with tile.TileContext(nc) as tc:
        dram = tc.alloc_tile_pool(name="dram", bufs=1, space="DRAM")
        q_sp = [dram.tile([HPC, 128, S], BF16, name=f"q_sp{b}")
                for b in range(B)]
        k_sp = [dram.tile([HPC, 128, S], BF16, name=f"k_sp{b}")
                for b in range(B)]
        a2a_in = [dram.tile([NCORES, F, 256], BF16, name=f"a2a_in{b}")
                  for b in range(B)]
        a2a_out = [dram.tile([NCORES, F, 256], BF16, name=f"a2a_out{b}")
                   for b in range(B)]

        with tc.tile_pool(name="consts", bufs=1) as cpool:
            perm_sb = cpool.tile([128, 128], BF16)
            nc.gpsimd.dma_start(out=perm_sb[:], in_=perm_d[:, :])
            onesb_sb = cpool.tile([128, 128], BF16)
            nc.gpsimd.dma_start(out=onesb_sb[:], in_=onesb_d[:, :])
            mask_sb = cpool.tile([128, 4 * 512], BF16)
            nc.gpsimd.dma_start(out=mask_sb[:], in_=mask_d[:, :])
            cos_sb = cpool.tile([128, S], BF16)
            nc.gpsimd.dma_start(out=cos_sb[:], in_=cos_d[:, :])
            sin_sb = cpool.tile([128, S], BF16)
            nc.gpsimd.dma_start(out=sin_sb[:], in_=sin_d[:, :])
            # v for both batches, all 4 heads: [128 tok-part, 32 blk * 512]
            v_all = cpool.tile([128, (TOK // 128) * F], BF16, name="v_all")

            # q/k tiles for the very first attention head, loaded mid-QKV
            q0_sb = cpool.tile([128, S], BF16, tag="q0", name="q0_sb")
            k0_sb = cpool.tile([128, S], BF16, tag="k0", name="k0_sb")

            # ======== merged QKV phase: one pass over x, weights resident
            with tc.tile_pool(name="wpool", bufs=1) as wpool, \
                 tc.tile_pool(name="xpool", bufs=2) as xpool, \
                 tc.tile_pool(name="qkps", bufs=4, space="PSUM") as qkps, \
                 tc.tile_pool(name="vps", bufs=2, space="PSUM") as vps, \
                 tc.tile_pool(name="rotps", bufs=2, space="PSUM") as rotps, \
                 tc.tile_pool(name="rwork", bufs=2) as rwork:

                w_sb = {}
                for nm, wd in (("q", wq_d), ("k", wk_d)):
                    t = wpool.tile([128, HPC * KT * 128], BF16, tag=f"w{nm}",
                                   name=f"w_{nm}")
                    ml = KT * 128
                    for m in range(HPC):
                        nc.scalar.dma_start(
                            out=t[:, m * ml:(m + 1) * ml],
                            in_=wd[:, m * ml:(m + 1) * ml])
                    w_sb[nm] = t
                wv_sb = wpool.tile([128, KT * F], BF16, tag="wv", name="w_v")
                chunk = KT * F // 8
                for c in range(8):
                    nc.scalar.dma_start(
                        out=wv_sb[:, c * chunk:(c + 1) * chunk],
                        in_=wv_d[:, c * chunk:(c + 1) * chunk])

                pending = []   # delayed rope: (raw, o_sp, m, nb)

                def emit_rope(item):
                    raw, o_sp, m, nb = item
                    rot = rotps.tile([128, 256], F32, tag="rot", name="rot")
                    nc.tensor.matmul(rot[:], perm_sb[:], raw[:],
                                     start=True, stop=True)
                    c0 = (nb % (S // 256)) * 256
                    t1 = rwork.tile([128, 256], BF16, tag="t1", name="t1")
                    nc.vector.tensor_mul(t1[:], raw[:], cos_sb[:, c0:c0 + 256])
                    t2 = rwork.tile([128, 256], BF16, tag="t2", name="t2")
                    nc.vector.tensor_mul(t2[:], rot[:], sin_sb[:, c0:c0 + 256])
                    qf = rwork.tile([128, 256], BF16, tag="qf", name="qf")
                    nc.vector.tensor_add(qf[:], t1[:], t2[:])
                    b, pos = (nb * 256) // S, (nb * 256) % S
                    nc.sync.dma_start(
                        out=o_sp[b][m, :, pos:pos + 256], in_=qf[:])

                for nb in range(NB):
                    xh = xpool.tile([128, KT * 256], BF16, tag="xh", name="xh")
                    xc = KT * 256 // 4
                    for c in range(4):
                        nc.sync.dma_start(
                            out=xh[:, c * xc:(c + 1) * xc],
                            in_=x_d[nb, :, c * xc:(c + 1) * xc])
                    for qk in range(2):
                        w_t = w_sb["q"] if qk == 0 else w_sb["k"]
                        o_sp = q_sp if qk == 0 else k_sp
                        for m in range(HPC):
                            ps = qkps.tile([128, 256], F32, tag="ps", name="ps")
                            for kt in range(KT):
                                nc.tensor.matmul(
                                    ps[:],
                                    w_t[:, (m * KT + kt) * 128:
                                        (m * KT + kt + 1) * 128],
                                    xh[:, kt * 256:(kt + 1) * 256],
                                    start=(kt == 0), stop=(kt == KT - 1))
                            raw = rwork.tile([128, 256], BF16, tag="raw",
                                             name="raw")
                            nc.scalar.copy(raw[:], ps[:])
                            pending.append((raw, o_sp, m, nb))
                            if len(pending) > 1:
                                emit_rope(pending.pop(0))
                    for mt in range(2):
                        vp = vps.tile([128, F], F32, tag="vp", name="vp")
                        for kt in range(KT):
                            nc.tensor.matmul(
                                vp[:],
                                xh[:, kt * 256 + mt * 128:
                                   kt * 256 + (mt + 1) * 128],
                                wv_sb[:, kt * F:(kt + 1) * F],
                                start=(kt == 0), stop=(kt == KT - 1))
                        st = nb * 2 + mt
                        nc.scalar.copy(v_all[:, st * F:(st + 1) * F], vp[:])
                    if nb == NB // 2 - 1:
                        # batch 0 q/k complete: prefetch head (0,0)
                        nc.scalar.dma_start(out=q0_sb[:],
                                            in_=q_sp[0][0, :, :])
                        nc.scalar.dma_start(out=k0_sb[:],
                                            in_=k_sp[0][0, :, :])
                while pending:
                    emit_rope(pending.pop(0))

            # ======== attention (woven) + per-batch AllToAll
            with tc.tile_pool(name="aqk", bufs=2) as apool, \
                 tc.tile_pool(name="exw", bufs=22) as expool, \
                 tc.tile_pool(name="amisc", bufs=3) as misc, \
                 tc.tile_pool(name="atprep", bufs=1) as atprep, \
                 tc.tile_pool(name="wo0p", bufs=1) as wo0p:

                at4 = [[atprep.tile([128, 8 * 256], BF16, tag=f"at4_{b}_{j}",
                                    name=f"at4_{b}_{j}")
                        for j in range(4)] for b in range(B)]
                wo0 = [wo0p.tile([128, 8 * 512], BF16, tag=f"wo0_{q}",
                                 name=f"wo0_{q}") for q in range(2)]

                heads = [(b, h) for b in range(B) for h in range(HPC)]
                tiles = {(0, 0): (q0_sb, k0_sb)}

                def emit_qk_load(bh):
                    b, h = bh
                    q_sb = apool.tile([128, S], BF16, tag="q", name="q_sb")
                    nc.scalar.dma_start(out=q_sb[:], in_=q_sp[b][h, :, :])
                    k_sb = apool.tile([128, S], BF16, tag="k", name="k_sb")
                    nc.scalar.dma_start(out=k_sb[:], in_=k_sp[b][h, :, :])
                    tiles[bh] = (q_sb, k_sb)

                with tc.tile_pool(name="scps", bufs=2, space="PSUM") as scps, \
                     tc.tile_pool(name="pvps", bufs=2, space="PSUM") as pvps, \
                     tc.tile_pool(name="dps", bufs=2, space="PSUM") as dps:

                    finq = []

                    def drain(k):
                        for _ in range(min(k, len(finq))):
                            finq.pop(0)()

                    def build_fins(b, h, qt, exs):
                        nkt = len(exs)
                        nds = nkt
                        st8 = b * SBLK
                        state = {}

                        def f_ds(i):
                            def go():
                                if i == 0:
                                    state["ds"] = dps.tile(
                                        [1, 512], F32, tag="dsum", name="dsum")
                                nc.tensor.matmul(
                                    state["ds"][:], onesb_sb[:, 0:1], exs[i],
                                    start=(i == 0), stop=(i == nds - 1))
                                if i == nds - 1:
                                    rec = misc.tile([1, 512], BF16, tag="rec",
                                                    name="rec")
                                    with nc.allow_low_precision(
                                            reason="softmax denom bf16"):
                                        nc.vector.reciprocal(
                                            rec[:], state["ds"][:])
                                    state["rec"] = rec
                            return go

                        def f_pv(i):
                            def go():
                                if i == 0:
                                    state["pv"] = pvps.tile(
                                        [128, 512], F32, tag="pv", name="pv")
                                blk = (st8 + i) * F + h * 128
                                nc.tensor.matmul(
                                    state["pv"][:],
                                    v_all[:, blk:blk + 128], exs[i],
                                    start=(i == 0), stop=(i == nkt - 1))
                            return go

                        def f_bc():
                            def go():
                                bc = dps.tile([128, 512], F32, tag="dsum",
                                              name="bc")
                                nc.tensor.matmul(bc[:], onesb_sb[0:1, :],
                                                 state["rec"][:],
                                                 start=True, stop=True)
                                bcs = misc.tile([128, 512], F32, tag="bcs",
                                                name="bcs")
                                nc.scalar.copy(bcs[:], bc[:])
                                state["bcs"] = bcs
                            return go

                        def f_at():
                            def go():
                                at = misc.tile([128, 512], BF16, tag="at",
                                               name="at")
                                nc.vector.tensor_mul(
                                    at[:], state["pv"][:], state["bcs"][:])
                                for u in range(2):
                                    nc.sync.dma_start(
                                        out=a2a_in[b][2 * qt + u,
                                                      h * 128:(h + 1) * 128, :],
                                        in_=at[:, u * 256:(u + 1) * 256])
                            return go

                        fins = [f_ds(i) for i in range(nds)]
                        fins += [f_pv(i) for i in range(nkt)]
                        fins += [f_bc(), f_at()]
                        return fins

                    def emit_at4(b):
                        for j in range(4):
                            for c in range(8):
                                kt = j * 8 + c
                                jj, off = kt // 4, (kt % 4) * 128
                                nc.gpsimd.dma_start(
                                    out=at4[b][j][:, c * 256:(c + 1) * 256],
                                    in_=a2a_out[b][jj, off:off + 128, :])

                    pace = [0]
                    for hi, (b, h) in enumerate(heads):
                        q_sb, k_sb = tiles.pop((b, h))
                        if hi + 1 < len(heads):
                            emit_qk_load(heads[hi + 1])
                        qts = range(4) if hi % 2 == 0 else range(3, -1, -1)
                        for qt in qts:
                            nkt = 4 * qt + 4
                            exs = []
                            for kp in range(nkt // 2):
                                sc = scps.tile([128, 1024], F32, tag="sc",
                                               name="sc")
                                for hf in range(2):
                                    kt = 2 * kp + hf
                                    nc.tensor.matmul(
                                        sc[:, hf * 512:(hf + 1) * 512],
                                        k_sb[:, kt * 128:(kt + 1) * 128],
                                        q_sb[:, qt * 512:(qt + 1) * 512],
                                        start=True, stop=True)
                                ex = expool.tile([128, 1024], BF16, tag="ex",
                                                 name="ex")
                                nc.scalar.activation(
                                    ex[:], sc[:],
                                    mybir.ActivationFunctionType.Exp,
                                    scale=SCALE)
                                # causal mask applied post-exp (0/1 multiply)
                                # on Pool, off the mm->exp critical path
                                for hf in range(2):
                                    r = 2 * kp + hf - 4 * qt
                                    if r >= 0:
                                        sl = ex[:, hf * 512:(hf + 1) * 512]
                                        nc.vector.tensor_mul(
                                            sl, sl,
                                            mask_sb[:, r * 512:(r + 1) * 512])
                                exs.append(ex[:, 0:512])
                                exs.append(ex[:, 512:1024])
                                # global pacing: 88 fins / 20 pairs per head
                                pace[0] += 22
                                nd = pace[0] // 5
                                pace[0] -= nd * 5
                                drain(nd)
                            finq.extend(build_fins(b, h, qt, exs))
                        if b == 1 and h == 1:
                            # at4(b0): Pool reaches this mid-b1, a2a0 done,
                            # no engine stall; lands well before WO(b0)
                            emit_at4(0)
                        if h == HPC - 1:
                            drain(len(finq))
                            pace[0] = 0
                            nc.gpsimd.collective_compute(
                                "AllToAll", mybir.AluOpType.bypass,
                                replica_groups=[list(range(NCORES))],
                                ins=[a2a_in[b][:]], outs=[a2a_out[b][:]])
                            if b == 1:
                                emit_at4(1)
                            if b == 0:
                                # first WO block's weights, no deps: load now
                                for q in range(2):
                                    nc.sync.dma_start(
                                        out=wo0[q][:], in_=wo_d[0, q, :, :])

                # ======== WO: out[tok_slice] = attn @ wo^T, batch 0 then 1
                with tc.tile_pool(name="wop", bufs=3) as wopool, \
                     tc.tile_pool(name="pswo", bufs=1, space="PSUM") as wops, \
                     tc.tile_pool(name="wout", bufs=3) as wout:
                    for b in range(B):
                        for n in range(D // 512):
                            pss = [wops.tile([128, 512], F32, tag=f"pw{mt}",
                                             name=f"ps{mt}") for mt in range(2)]
                            for quad in range(4):
                                if b == 0 and n == 0 and quad < 2:
                                    wo_sb = wo0[quad]
                                else:
                                    wo_sb = wopool.tile([128, 8 * 512], BF16,
                                                        tag="wo", name="wo_sb")
                                    nc.scalar.dma_start(out=wo_sb[:],
                                                        in_=wo_d[n, quad, :, :])
                                for mt in range(2):
                                    for kk in range(8):
                                        kt = quad * 8 + kk
                                        nc.tensor.matmul(
                                            pss[mt][:],
                                            at4[b][kt // 8][
                                                :, (kt % 8) * 256 + mt * 128:
                                                (kt % 8) * 256 + (mt + 1) * 128],
                                            wo_sb[:, kk * 512:(kk + 1) * 512],
                                            start=(kt == 0), stop=(kt == KT - 1))
                            for mt in range(2):
                                o_sb = wout.tile([128, 512], F32, name="o_sb")
                                nc.scalar.copy(o_sb[:], pss[mt][:])
                                nc.sync.dma_start(
                                    out=out_d[b * 256 + mt * 128:
                                              b * 256 + (mt + 1) * 128,
                                              n * 512:(n + 1) * 512],
                                    in_=o_sb[:])

    nc.compile()
    return nc


def _host_inputs(x, wq, wk, wv, wo):
    bf = ml_dtypes.bfloat16
    x = np.asarray(x, dtype=np.float32).reshape(TOK, D)
    xT = x.T                                              # [D, TOK]
    xt = np.ascontiguousarray(
        xT.reshape(KT, 128, NB, 256).transpose(2, 1, 0, 3)
        .reshape(NB, 128, KT * 256)).astype(bf)

    woT = np.asarray(wo, dtype=np.float32).T              # [f_in, d_out]
    wot = np.ascontiguousarray(
        woT.reshape(4, 8, 128, 8, 512).transpose(3, 0, 2, 1, 4)
        .reshape(8, 4, 128, 8 * 512)).astype(bf)

    inv = (1.0 / (10000.0 ** (np.arange(0, HD, 2, dtype=np.float64) / HD)))
    fr = np.outer(np.arange(S, dtype=np.float64), inv)    # [S, HD/2]
    cosE = np.ascontiguousarray(
        np.repeat(np.cos(fr).T, 2, axis=0)).astype(bf)   # [128, S]
    sinE = np.ascontiguousarray(
        np.repeat(np.sin(fr).T, 2, axis=0)).astype(bf)

    masks = np.ones([128, 4 * 512], dtype=np.float32)
    qi = np.arange(512)
    pi = np.arange(128)
    for r in range(4):
        masks[:, r * 512:(r + 1) * 512][qi[None, :] < (r * 128 + pi)[:, None]] = 0.0

    permT = np.zeros([128, 128], dtype=np.float32)
    ii = np.arange(0, 128, 2)
    permT[ii + 1, ii] = -1.0
    permT[ii, ii + 1] = 1.0

    ones = np.ones([128, 128], dtype=np.float32)

    def wprep_m(w, sl):      # m-major: [128, HPC, KT, 128]
        wT = np.asarray(w, np.float32)[sl, :].T           # [D, F]
        return np.ascontiguousarray(
            wT.reshape(KT, 128, HPC, 128).transpose(1, 2, 0, 3)
            .reshape(128, HPC * KT * 128)).astype(bf)

    def wprep_kt(w, sl):     # kt-major: [128, KT, F]
        wT = np.asarray(w, np.float32)[sl, :].T
        return np.ascontiguousarray(
            wT.reshape(KT, 128, F).transpose(1, 0, 2)
            .reshape(128, KT * F)).astype(bf)

    maps = []
    for i in range(NCORES):
        sl = slice(i * F, (i + 1) * F)
        maps.append(dict(
            xt=xt,
            wqT=wprep_m(wq, sl),
            wkT=wprep_m(wk, sl),
            wvT=wprep_kt(wv, sl),
            woT=wot,
            cosE=cosE, sinE=sinE, masks=masks.astype(bf),
            permT=permT.astype(bf), onesb=ones.astype(bf),
        ))
    return maps


def kernel(x, start_pos, wq, wk, wv, wo, _trace=False):
    if "nc" not in _CACHE:
        _CACHE["nc"] = _build()
    nc = _CACHE["nc"]
    maps = _host_inputs(x, wq, wk, wv, wo)
    res = run_bass_kernel_spmd(nc, maps, core_ids=list(range(NCORES)),
                               trace=_trace)
    _CACHE["last"] = res
    full = np.empty([TOK, D], dtype=np.float32)
    for j in range(NCORES):
        o = res.results[j]["out"]
        full[j * 256:(j + 1) * 256] = o[:256]
        full[S + j * 256: S + (j + 1) * 256] = o[256:]
    return full.reshape(B, S, D)
